# revision 1
# baseline (speedup 1.0000x reference)
"""ABC-Conv (binary conv, 3 estimators) on 8 trn2 NeuronCores — fp8 DoubleRow.

Math: reference computes
    xq   = sign(x)
    beta = boxfilter3x3(sum_c |x|) / 1152                [B,110,110]
    out  = sum_e conv(xq, sign(kernels[e])) * beta[...,None] * alphas[e]

conv is linear in its kernel, so the estimator loop folds into ONE conv with
W = sum_e sign(kernels[e]) * alphas[e].  W is quantized to fp8 e4m3 with a
per-output-channel scale s[f] chosen by grid search to minimize ||q(sW)/s - W||
(measured end-to-end rel err 5.7e-3 vs the 2e-2 gate); the conv then runs as
fp8 DoubleRow matmuls (2 taps contracted per pass).

Sharding: data-parallel over batch, 2 images per core, weights replicated.

Per-core kernel layout (F-major conv):
  - host ships xT bf16 [128cin, 2*12768] (transposed, 2 zero-pad rows/img) and
    x_pm bf16 [128pix, 2*98, 128cin] (pixel-tile-major, for |x| channel sums)
  - sign on ScalarE -> xqT fp8 [cin, flatpix]
  - conv: stationary = W8 pair [cin, 2tap, 128f], moving = xqT pair
    [cin, 2tap, 256pix] (overlapping shifted AP planes), DoubleRow, psum
    [128f, 2fh, 256pix]; 10 matmuls per 256-px block form one psum chain
  - groups of 3 blocks reuse each stationary 3x to amortize LDWEIGHTS
  - beta: DVE abs-channel-sums -> 3 box matmuls (host-built 0/1 shift
    matrices) -> PE-transpose -> partition-collapse DMA -> row betaR ->
    gpsimd partition_broadcast -> betab [128, pix] f16
  - drain: psum * sinv[f] (per-partition scale, DVE/ScalarE alternating)
    -> bf16, then * betab (DVE, fh-broadcast) -> out [2img, 2fh, 128f, pix]
  - out pixel trick: flat p = h*112 + w; cols w in {110,111} and rows >= 110
    are garbage, sliced on host.
"""

import sys

sys.path.insert(0, "/opt/trn_rl_repo")

import bass_rust
import ml_dtypes
import numpy as np

import concourse.tile as tile
from concourse import bacc, mybir
from concourse.bass import ds
from concourse.bass_utils import run_bass_kernel_spmd

F32 = mybir.dt.float32
F16 = mybir.dt.float16
BF16 = mybir.dt.bfloat16
FP8 = mybir.dt.float8e4

N_CORES = 8
B_PER_CORE = 2
H = W_IMG = 112
CIN = 128
F = 256
E = 3
D_DIM = 9 * CIN  # 1152

IMG_PIX = H * W_IMG          # 12544
IMG_PAD = (H + 2) * W_IMG    # 12768 (2 zero rows terminate window reads)
OUT_ROWS = 110
OUT_PIX = OUT_ROWS * W_IMG   # 12320
N_OUT_TILES = 97             # 96 full 128-px tiles + one 32-row tile
STAGE_PIX = N_OUT_TILES * 128  # 12416 staged px per img (tail padded)
S_SEG = 100                  # s columns per img (98 real + 2 zero pad)
S_TILES = 98

DOFF = [kh * W_IMG + kw for kh in range(3) for kw in range(3)]
# DoubleRow plane strides must be %16==0 (HW constraint, probed).  A +1-px
# shifted alias of xqT (xqT2, built by sbuf->sbuf DMA at span offset XQ2) makes
# (d, d+1) pairs legal; (2,114) pairs across rows (stride 112); tap (2,2) is a
# stride-0 self-pair carrying a two-term fp8 expansion (Wa=q(sW), Wb=q(sW-Wa)).
XQ2 = B_PER_CORE * IMG_PAD  # 25536, %16==0
PAIR_OFF = [(0, XQ2), (2, 114), (112, XQ2 + 112), (224, XQ2 + 224), (226, 226)]
NPAIR = 5
# w8 slot order (2i, 2i+1) = the tap (kh,kw) each plane multiplies
SLOT_TAPS = [(0, 0), (0, 1), (0, 2), (1, 2), (1, 0), (1, 1), (2, 0), (2, 1)]

NBLK = 256               # pixels per conv block (= moving N per matmul)
GROUP = 3                # blocks per group (stationary reuse factor)
N_FULL_BLOCKS = 48       # 48*256 = 12288 px; tail block = 128 px
N_GROUPS = 16            # full groups per image
BSEG = ((0, 9), (9, 16), (25, 25), (50, 25), (75, 22))  # beta segments (tiles)

XT_CH = [(0, 1024), (1024, 1024), (2048, 2048), (4096, 2048), (6144, 2048),
         (8192, 2048), (10240, 2528)]  # covers IMG_PAD
XPM_CH = [(i * 14, 14) for i in range(7)]  # 98 s-tiles in 14-col chunks


def _box_matrices():
    """beta_pre[p, t] = sum_q sum_k Mq[k,p] * s[k, t+q]; window offsets reach
    p+353, spanning three 128-columns of s."""
    ms = np.zeros((3, 128, 128), np.float32)
    for p in range(128):
        for d in DOFF:
            k = p + d
            ms[k // 128, k % 128, p] = 1.0
    return ms.astype(ml_dtypes.bfloat16)


def _pair_view(ap_full, da, db, n):
    """[128, 2, n] AP over a [128, X] sbuf tile: planes at free offsets da, db."""
    dim0 = list(ap_full.ap[0])
    return bass_rust.AP(
        ap_full.tensor, ap_full.offset + da, [dim0, [db - da, 2], [1, n]]
    )


def build_nc():
    nc = bacc.Bacc("TRN2", target_bir_lowering=False, debug=False)
    xt_d = nc.dram_tensor("xT", [CIN, B_PER_CORE * IMG_PAD], BF16, kind="ExternalInput").ap()
    xpm_d = nc.dram_tensor("xpm", [128, B_PER_CORE * S_TILES, CIN], BF16, kind="ExternalInput").ap()
    w_d = nc.dram_tensor("w8", [CIN, 2 * NPAIR * F], FP8, kind="ExternalInput").ap()
    m_d = nc.dram_tensor("boxm", [3, 128, 128], BF16, kind="ExternalInput").ap()
    o_d = nc.dram_tensor("out", [B_PER_CORE * 2 * 128, STAGE_PIX], BF16, kind="ExternalOutput").ap()

    with tile.TileContext(nc) as tc:
        with (
            tc.tile_pool(name="const", bufs=1) as constp,
            tc.tile_pool(name="big", bufs=1) as bigp,
            tc.tile_pool(name="xin", bufs=3) as xinp,
            tc.tile_pool(name="xpm", bufs=3) as xpmp,
            tc.tile_pool(name="o2", bufs=3) as o2p,
            tc.tile_pool(name="betab", bufs=1) as betabp,
            tc.tile_pool(name="psum", bufs=7, space="PSUM") as psump,
            tc.tile_pool(name="psb", bufs=1, space="PSUM") as psbp,
        ):
            # ---------- constants ----------
            boxm = constp.tile([128, 3, 128], BF16)
            nc.gpsimd.dma_start(boxm[:, :, :], m_d.rearrange("m k p -> k m p"))
            w8 = constp.tile([128, 2 * NPAIR, F], FP8)
            nc.sync.dma_start(w8[:, :, :], w_d.rearrange("c (t f) -> c t f", t=2 * NPAIR))

            # ---------- persistent buffers ----------
            xqT = bigp.tile([128, 2 * B_PER_CORE * IMG_PAD], FP8)  # [xqT | xqT2]
            s_f = bigp.tile([128, B_PER_CORE * S_SEG], F32)
            s_bf = bigp.tile([128, B_PER_CORE * S_SEG], BF16)
            betaT = bigp.tile([128, B_PER_CORE * 5 * 128], F16)  # [tile, seg slot]
            betaR = bigp.tile([1, B_PER_CORE * STAGE_PIX], F16)
            for b in range(B_PER_CORE):
                nc.vector.memset(s_bf[:, ds(b * S_SEG + S_TILES, 2)], 0.0)

            betab0 = betabp.tile([128, STAGE_PIX], F16, tag="betab0")
            betab1 = betabp.tile([128, STAGE_PIX], F16, tag="betab1")
            betab = [betab0, betab1]

            store_q = [nc.sync, nc.scalar]
            nstore = 0
            shift_pos = [b * IMG_PAD for b in range(B_PER_CORE)]

            def emit_xpm_chunk(b, t0, nt):
                xst = xpmp.tile([128, 14, CIN], BF16, tag="xpst")
                nc.scalar.dma_start(
                    xst[:, :nt, :], xpm_d[:, ds(b * S_TILES + t0, nt), :]
                )
                nc.vector.tensor_reduce(
                    s_f[:, ds(b * S_SEG + t0, nt)],
                    xst[:, :nt, :],
                    axis=mybir.AxisListType.X,
                    op=mybir.AluOpType.add,
                    apply_absolute_value=True,
                )
                nc.scalar.copy(
                    s_bf[:, ds(b * S_SEG + t0, nt)], s_f[:, ds(b * S_SEG + t0, nt)]
                )

            def emit_xt_chunk(b, c0, npix):
                xst = xinp.tile([128, 2560], BF16, tag="xtst")
                nc.sync.dma_start(
                    xst[:, :npix], xt_d[:, ds(b * IMG_PAD + c0, npix)]
                )
                nc.scalar.sign(
                    xqT[:, ds(b * IMG_PAD + c0, npix)], xst[:, :npix]
                )
                # extend the +1-shifted alias (xqT2) as far as signed data allows
                new_end = b * IMG_PAD + c0 + npix - 1
                ln = new_end - shift_pos[b]
                nc.sync.dma_start(
                    xqT[:, ds(XQ2 + shift_pos[b], ln)],
                    xqT[:, ds(shift_pos[b] + 1, ln)],
                )
                shift_pos[b] = new_end

            def emit_beta_seg(b, seg, c0, cn):
                # box filter emitted directly transposed: lhsT = s segment, so
                # psum comes out [tile, 128] = betaT layout
                btp = psbp.tile([32, 128], F32, tag="btp")
                for q in range(3):
                    nc.tensor.matmul(
                        btp[:cn, :],
                        lhsT=s_bf[:, ds(b * S_SEG + c0 + q, cn)],
                        rhs=boxm[:, q, :],
                        start=(q == 0),
                        stop=(q == 2),
                    )
                slot = ds((b * 5 + seg) * 128, 128)
                nc.vector.tensor_scalar_mul(betaT[:cn, slot], btp[:cn, :], 1.0 / D_DIM)
                # partition-collapse to a row (3D dst keeps partition dim 1)
                nc.sync.dma_start(
                    betaR[:, ds(b * STAGE_PIX + c0 * 128, cn * 128)].rearrange(
                        "o (t p) -> o t p", t=cn
                    ),
                    betaT[:cn, slot],
                )
                nc.gpsimd.partition_broadcast(
                    betab[b][:, ds(c0 * 128, cn * 128)],
                    betaR[:, ds(b * STAGE_PIX + c0 * 128, cn * 128)],
                )

            def emit_conv_group(b, g, blocks):
                nonlocal nstore
                npx = [NBLK if 3 * g + k < N_FULL_BLOCKS else 128 for k in range(blocks)]
                ps = [
                    psump.tile([128, 2, NBLK], F32, tag="ps", name=f"ps{g}_{k}")
                    for k in range(blocks)
                ]
                base = b * IMG_PAD + (3 * g) * NBLK
                for i, (da, db) in enumerate(PAIR_OFF):
                    for fh in range(2):
                        for k in range(blocks):
                            nc.tensor.matmul(
                                ps[k][:, fh, : npx[k]],
                                lhsT=w8[:, ds(2 * i, 2), ds(fh * 128, 128)],
                                rhs=_pair_view(
                                    xqT[:, :], base + k * NBLK + da, base + k * NBLK + db, npx[k]
                                ),
                                start=(i == 0 and fh == 0),
                                stop=(i == NPAIR - 1 and fh == 1),
                                perf_mode=mybir.MatmulPerfMode.DoubleRow,
                            )
                o2t = o2p.tile([128, 2, GROUP * NBLK], BF16, tag="o2")
                px0 = (3 * g) * NBLK
                for k in range(blocks):
                    bb = (
                        betab[b][:, ds(px0 + k * NBLK, npx[k])]
                        .unsqueeze(1)
                        .broadcast_to([128, 2, npx[k]])
                    )
                    nc.vector.tensor_mul(
                        o2t[:, :, ds(k * NBLK, npx[k])], ps[k][:, :, : npx[k]], bb
                    )
                tot = sum(npx)
                store_q[nstore % 2].dma_start(
                    o_d[ds(b * 256, 256), ds(px0, tot)].rearrange(
                        "(fh p) x -> p fh x", fh=2
                    ),
                    o2t[:, :, :tot],
                )
                nstore += 1

            # ---------- interleaved two-image pipeline ----------
            bq = [0] * B_PER_CORE
            gt = [0] * B_PER_CORE
            betab_px = [0] * B_PER_CORE
            scols_v = [0] * B_PER_CORE
            pcov_v = [0] * B_PER_CORE
            for c in range(7):
                for b in range(B_PER_CORE):
                    t0, nt = XPM_CH[c]
                    emit_xpm_chunk(b, t0, nt)
                    scols_v[b] = S_SEG if c == 6 else 14 * (c + 1)
                    c0, npix = XT_CH[c]
                    emit_xt_chunk(b, c0, npix)
                    pcov_v[b] = c0 + npix
                for b in range(B_PER_CORE):
                    while (
                        bq[b] < len(BSEG)
                        and BSEG[bq[b]][0] + BSEG[bq[b]][1] + 2 <= scols_v[b]
                    ):
                        emit_beta_seg(b, bq[b], *BSEG[bq[b]])
                        betab_px[b] = (BSEG[bq[b]][0] + BSEG[bq[b]][1]) * 128
                        bq[b] += 1
                progressed = True
                while progressed:
                    progressed = False
                    for b in range(B_PER_CORE):
                        if gt[b] >= N_GROUPS + 1:
                            continue
                        if gt[b] < N_GROUPS:
                            need_px = 768 * gt[b] + 995
                            need_bb = 768 * (gt[b] + 1)
                            blocks = 3
                        else:
                            need_px = IMG_PAD
                            need_bb = STAGE_PIX
                            blocks = 1
                        if need_px <= pcov_v[b] and need_bb <= betab_px[b]:
                            emit_conv_group(b, gt[b], blocks)
                            gt[b] += 1
                            progressed = True
            assert bq == [len(BSEG)] * B_PER_CORE, bq
            assert gt == [N_GROUPS + 1] * B_PER_CORE, gt

    nc.compile()
    return nc


_NC = None


def _get_nc():
    global _NC
    if _NC is None:
        _NC = build_nc()
    return _NC


def _quantize_weights(kernels, alphas):
    """Fold estimators, then per-channel-scale fp8 e4m3 quantization."""
    sgn = np.where(kernels >= 0, 1.0, -1.0).astype(np.float32)  # [E,3,3,128,256]
    W = np.einsum("ehwcf,ef->hwcf", sgn, alphas.astype(np.float32))  # [3,3,128,256]
    # scale search on the single-term taps; tap (2,2) is two-term (near exact)
    Wf = np.stack([W[kh, kw] for kh, kw in SLOT_TAPS]).reshape(8 * CIN, F)
    scales = np.geomspace(6.0, 100.0, 385).astype(np.float32)
    q = (Wf[None, :, :] * scales[:, None, None]).astype(ml_dtypes.float8_e4m3fn)
    err = ((q.astype(np.float32) / scales[:, None, None] - Wf[None]) ** 2).sum(axis=1)
    s = scales[np.argmin(err, axis=0)]  # [F]
    Wq = (W * s).astype(ml_dtypes.float8_e4m3fn)  # [3,3,128,256]
    Wres = W * s - Wq.astype(np.float32)
    Wq2 = Wres.astype(ml_dtypes.float8_e4m3fn)  # second term for tap (2,2)
    # slot pairs (2i, 2i+1) follow PAIR_OFF via SLOT_TAPS; tap (2,2) two-term
    w8 = np.zeros((CIN, 2 * NPAIR, F), ml_dtypes.float8_e4m3fn)
    for j, (kh, kw) in enumerate(SLOT_TAPS):
        w8[:, j, :] = Wq[kh, kw]
    w8[:, 8, :] = Wq[2, 2]
    w8[:, 9, :] = Wq2[2, 2]
    sinv = (1.0 / s).astype(np.float32)  # [F]; applied on host after gather
    return np.ascontiguousarray(w8.reshape(CIN, 2 * NPAIR * F)), sinv


def _in_maps(x, kernels, alphas):
    x = np.asarray(x, np.float32)
    kernels = np.asarray(kernels, np.float32)
    alphas = np.asarray(alphas, np.float32)
    w8, sinv = _quantize_weights(kernels, alphas)
    boxm = _box_matrices()

    xb = x.astype(ml_dtypes.bfloat16)  # sign-exact; |x| sums lose <0.1%
    xs = xb.reshape(N_CORES, B_PER_CORE, IMG_PIX, CIN)
    maps = []
    for c in range(N_CORES):
        xT = np.zeros((CIN, B_PER_CORE * IMG_PAD), ml_dtypes.bfloat16)
        for b in range(B_PER_CORE):
            xT[:, b * IMG_PAD : b * IMG_PAD + IMG_PIX] = xs[c, b].T
        xpm = np.ascontiguousarray(
            xs[c].reshape(B_PER_CORE, S_TILES, 128, CIN)
            .transpose(2, 0, 1, 3)
            .reshape(128, B_PER_CORE * S_TILES, CIN)
        )
        maps.append(
            {
                "xT": np.ascontiguousarray(xT),
                "xpm": xpm,
                "w8": w8,
                "boxm": boxm,
            }
        )
    return maps, sinv


def _gather(results, sinv):
    outs = []
    sv = sinv.reshape(1, F, 1, 1)
    for c in range(N_CORES):
        o = np.asarray(results[c]["out"]).astype(np.float32)  # [512, 12416]
        o = o.reshape(B_PER_CORE, F, STAGE_PIX)[:, :, :OUT_PIX]
        o = o.reshape(B_PER_CORE, F, OUT_ROWS, W_IMG)[:, :, :, :OUT_ROWS] * sv
        outs.append(o.transpose(0, 2, 3, 1))
    return np.ascontiguousarray(np.concatenate(outs, axis=0))


def kernel(x, kernels, alphas):
    nc = _get_nc()
    maps, sinv = _in_maps(x, kernels, alphas)
    res = run_bass_kernel_spmd(nc, maps, core_ids=list(range(N_CORES)))
    return _gather(res.results, sinv)


def _install_profile_hook():
    """The agent image's antenv lacks axon_hooks; recreate it so
    run_bass_kernel_spmd(trace=True) can NTFF-profile via libaxon_pjrt.so."""
    import types

    import antenv

    if "antenv.axon_hooks" in sys.modules:
        return
    mod = types.ModuleType("antenv.axon_hooks")
    holder = {}
    mod.set_axon_ntff_profile_hook = lambda h: holder.__setitem__("h", h)
    mod.get_axon_ntff_profile_hook = lambda: holder.get("h")
    sys.modules["antenv.axon_hooks"] = mod
    antenv.axon_hooks = mod

    from trn_agent_boot.trn_boot import _ntff_profile_via_ctypes

    hook = _ntff_profile_via_ctypes("/opt/axon/libaxon_pjrt.so")
    mod.set_axon_ntff_profile_hook(hook)

    # upload_artifacts wants a cloud bucket; keep everything local instead.
    import concourse.bass_utils as bu

    bu.upload_artifacts = lambda tmpdir: tmpdir


def run_profiled(x, kernels, alphas, tmpdir=None):
    """Returns (output, exec_time_ns, profile_json_path)."""
    _install_profile_hook()
    nc = _get_nc()
    maps, sinv = _in_maps(x, kernels, alphas)
    res = run_bass_kernel_spmd(
        nc,
        maps,
        core_ids=list(range(N_CORES)),
        trace=True,
        tmpdir=tmpdir,
    )
    return _gather(res.results, sinv), res.exec_time_ns, res.profile_json



# revision 6
# speedup vs baseline: 1.0374x; 1.0374x over previous
"""ABC-Conv (binary conv, 3 estimators) on 8 trn2 NeuronCores — fp8 DoubleRow,
transposed-output layout.

Math: reference computes
    xq   = sign(x)
    beta = boxfilter3x3(sum_c |x|) / 1152                [B,110,110]
    out  = sum_e conv(xq, sign(kernels[e])) * beta[...,None] * alphas[e]

conv is linear in its kernel, so the estimator loop folds into ONE conv with
W = sum_e sign(kernels[e]) * alphas[e].  W is quantized to fp8 e4m3 with a
per-output-channel scale s[f] (grid search); the conv runs as fp8 DoubleRow
matmuls (2 taps contracted per pass).

Transposed layout (vs the earlier F-major design): stationary = xq pixel-tile
pair planes [cin, 2tap, 128px], moving = w8 [cin, 2tap, 256f], psum
[128px, 256f].  This puts the output PIXEL in the partition dim, so the
per-pixel beta scale becomes a per-partition scalar: the psum drain is a
single activation/tensor_scalar multiply that can alternate between ScalarE
and DVE, and the whole betab broadcast pipeline (partition-collapse DMA +
gpsimd partition_broadcast of [128, 12416] tiles) disappears.  beta is
consumed directly in [pos-in-tile, tile] orientation (betaCol), which the box
matmuls emit natively when run with boxm as the stationary operand.

Sharding: data-parallel over batch, 2 images per core, weights replicated.

Per-core pipeline:
  - host ships xT bf16 [128cin, 2*12768] (2 zero-pad rows/img) and
    x_pm bf16 [128pix, 2*98, 128cin] (pixel-tile-major, for |x| channel sums)
  - sign on ScalarE -> xqT fp8 [cin, flatpix]; +1-shifted alias (xqT2) built
    by sbuf->sbuf DMA so DoubleRow pair plane strides hit the %16 constraint
  - conv: per 2 px-tiles, one 10-matmul DoubleRow chain into a full psum bank
    [128px, 2tile, 256f] (first start=True zeroes the bank)
  - beta: DVE abs-channel-sums (7-col slices) -> 3 box matmuls per 8-tile
    segment with boxm stationary -> betaCol [128pos, tile] f32 (ScalarE scale)
  - drain: per px-tile, psum * betaCol[:, t] (per-partition scale) -> bf16,
    alternating ScalarE activation / DVE tensor_scalar; 4 tiles per store DMA
    (sync / gpsimd queues alternate)
  - out [img, tile, px, f]; garbage cols w in {110,111} and rows >= 110
    sliced on host; sinv[f] (fp8 scale inverse) applied on host.
"""

import sys

sys.path.insert(0, "/opt/trn_rl_repo")

import bass_rust
import ml_dtypes
import numpy as np

import concourse.tile as tile
from concourse import bacc, mybir
from concourse.bass import ds
from concourse.bass_utils import run_bass_kernel_spmd

F32 = mybir.dt.float32
F16 = mybir.dt.float16
BF16 = mybir.dt.bfloat16
FP8 = mybir.dt.float8e4

N_CORES = 8
B_PER_CORE = 2
H = W_IMG = 112
CIN = 128
F = 256
E = 3
D_DIM = 9 * CIN  # 1152

IMG_PIX = H * W_IMG          # 12544
IMG_PAD = (H + 2) * W_IMG    # 12768 (2 zero rows terminate window reads)
OUT_ROWS = 110
OUT_PIX = OUT_ROWS * W_IMG   # 12320
N_TILES = 97                 # 128-px output tiles per image (96 full + tail)
STAGE_PIX = N_TILES * 128    # 12416 staged px per img (tail padded)
S_SEG = 100                  # s columns per img (98 real + 2 zero pad)
S_TILES = 98

# DoubleRow plane strides must be %16==0 (HW constraint, probed).  A +1-px
# shifted alias of xqT (xqT2, built by sbuf->sbuf DMA at span offset XQ2) makes
# (d, d+1) pairs legal; (2,114) pairs across rows (stride 112); tap (2,2) is a
# stride-0 self-pair carrying a two-term fp8 expansion (Wa=q(sW), Wb=q(sW-Wa)).
XQ2 = B_PER_CORE * IMG_PAD  # 25536, %16==0
PAIR_OFF = [(0, XQ2), (2, 114), (112, XQ2 + 112), (224, XQ2 + 224), (226, 226)]
NPAIR = 5
# w8 slot order (2i, 2i+1) = the tap (kh,kw) each plane multiplies
SLOT_TAPS = [(0, 0), (0, 1), (0, 2), (1, 2), (1, 0), (1, 1), (2, 0), (2, 1)]

N_PAIRS = 49                 # px-tile pairs per image (48 full + single tail)
STG = 4                      # px-tiles per output store DMA
BSEGS = [(8 * k, 8) for k in range(12)] + [(96, 1)]  # betaCol segments

XT_CH = [(0, 1024), (1024, 1024), (2048, 2048), (4096, 2048), (6144, 2048),
         (8192, 2048), (10240, 2528)]  # covers IMG_PAD
XPM_CH = [(i * 14, 14) for i in range(7)]  # 98 s-tiles in 14-col chunks


def _box_matrices():
    """beta_pre[p, t] = sum_q sum_k Mq[k,p] * s[k, t+q]; window offsets reach
    p+353, spanning three 128-columns of s."""
    doff = [kh * W_IMG + kw for kh in range(3) for kw in range(3)]
    ms = np.zeros((3, 128, 128), np.float32)
    for p in range(128):
        for d in doff:
            k = p + d
            ms[k // 128, k % 128, p] = 1.0
    return ms.astype(ml_dtypes.bfloat16)


def _pair_view(ap_full, da, db, n):
    """[128, 2, n] AP over a [128, X] sbuf tile: planes at free offsets da, db."""
    dim0 = list(ap_full.ap[0])
    return bass_rust.AP(
        ap_full.tensor, ap_full.offset + da, [dim0, [db - da, 2], [1, n]]
    )


def build_nc():
    nc = bacc.Bacc("TRN2", target_bir_lowering=False, debug=False)
    xt_d = nc.dram_tensor("xT", [CIN, B_PER_CORE * IMG_PAD], BF16, kind="ExternalInput").ap()
    xpm_d = nc.dram_tensor("xpm", [128, B_PER_CORE * S_TILES, CIN], BF16, kind="ExternalInput").ap()
    w_d = nc.dram_tensor("w8", [CIN, 2 * NPAIR * F], FP8, kind="ExternalInput").ap()
    m_d = nc.dram_tensor("boxm", [3, 128, 128], BF16, kind="ExternalInput").ap()
    o_d = nc.dram_tensor("out", [B_PER_CORE * N_TILES * 128, F], BF16, kind="ExternalOutput").ap()

    with tile.TileContext(nc) as tc:
        with (
            tc.tile_pool(name="const", bufs=1) as constp,
            tc.tile_pool(name="big", bufs=1) as bigp,
            tc.tile_pool(name="xin", bufs=3) as xinp,
            tc.tile_pool(name="xpm", bufs=3) as xpmp,
            tc.tile_pool(name="o2", bufs=3) as o2p,
            tc.tile_pool(name="psum", bufs=7, space="PSUM") as psump,
            tc.tile_pool(name="psb", bufs=1, space="PSUM") as psbp,
        ):
            # ---------- constants ----------
            boxm = constp.tile([128, 3, 128], BF16)
            nc.gpsimd.dma_start(boxm[:, :, :], m_d.rearrange("m k p -> k m p"))
            w8 = constp.tile([128, 2 * NPAIR, F], FP8)
            nc.sync.dma_start(w8[:, :, :], w_d.rearrange("c (t f) -> c t f", t=2 * NPAIR))

            # ---------- persistent buffers ----------
            xqT = bigp.tile([128, 2 * B_PER_CORE * IMG_PAD], FP8)  # [xqT | xqT2]
            s_f = bigp.tile([128, B_PER_CORE * S_SEG], F32)
            s_bf = bigp.tile([128, B_PER_CORE * S_SEG], BF16)
            betaCol = bigp.tile([128, B_PER_CORE * N_TILES], F32)
            for b in range(B_PER_CORE):
                nc.vector.memset(s_bf[:, ds(b * S_SEG + S_TILES, 2)], 0.0)

            shift_pos = [b * IMG_PAD for b in range(B_PER_CORE)]
            ndrain = 0
            nstore = 0
            store_q = [nc.sync, nc.gpsimd]

            def emit_xpm_chunk(b, t0, nt):
                xst = xpmp.tile([128, 14, CIN], BF16, tag="xpst")
                nc.gpsimd.dma_start(
                    xst[:, :nt, :], xpm_d[:, ds(b * S_TILES + t0, nt), :]
                )
                # two reduce slices for finer DVE interleave with drains
                for h0, hn in ((0, nt // 2), (nt // 2, nt - nt // 2)):
                    nc.vector.tensor_reduce(
                        s_f[:, ds(b * S_SEG + t0 + h0, hn)],
                        xst[:, ds(h0, hn), :],
                        axis=mybir.AxisListType.X,
                        op=mybir.AluOpType.add,
                        apply_absolute_value=True,
                    )
                nc.scalar.copy(
                    s_bf[:, ds(b * S_SEG + t0, nt)], s_f[:, ds(b * S_SEG + t0, nt)]
                )

            def emit_xt_chunk(b, c0, npix):
                xst = xinp.tile([128, 2560], BF16, tag="xtst")
                nc.sync.dma_start(
                    xst[:, :npix], xt_d[:, ds(b * IMG_PAD + c0, npix)]
                )
                nc.scalar.sign(
                    xqT[:, ds(b * IMG_PAD + c0, npix)], xst[:, :npix]
                )
                # extend the +1-shifted alias (xqT2) as far as signed data allows
                new_end = b * IMG_PAD + c0 + npix - 1
                ln = new_end - shift_pos[b]
                nc.sync.dma_start(
                    xqT[:, ds(XQ2 + shift_pos[b], ln)],
                    xqT[:, ds(shift_pos[b] + 1, ln)],
                )
                shift_pos[b] = new_end

            def emit_beta_seg(b, t0, cn):
                # box filter with boxm stationary: psum comes out [pos, tile]
                btp = psbp.tile([128, 8], F32, tag="btp")
                for q in range(3):
                    nc.tensor.matmul(
                        btp[:, :cn],
                        lhsT=boxm[:, q, :],
                        rhs=s_bf[:, ds(b * S_SEG + t0 + q, cn)],
                        start=(q == 0),
                        stop=(q == 2),
                    )
                nc.scalar.mul(
                    betaCol[:, ds(b * N_TILES + t0, cn)], btp[:, :cn], 1.0 / D_DIM
                )

            def emit_mm_pair(b, p):
                ntile = 2 if p < N_PAIRS - 1 else 1
                ps = psump.tile([128, 2, F], F32, tag="ps", name=f"ps{b}_{p}")
                base = b * IMG_PAD + (2 * p) * 128
                nmm = NPAIR * ntile
                k = 0
                for j in range(ntile):
                    for i, (da, db) in enumerate(PAIR_OFF):
                        nc.tensor.matmul(
                            ps[:, j, :],
                            lhsT=_pair_view(
                                xqT[:, :], base + j * 128 + da, base + j * 128 + db, 128
                            ),
                            rhs=w8[:, ds(2 * i, 2), :],
                            start=(k == 0),
                            stop=(k == nmm - 1),
                            perf_mode=mybir.MatmulPerfMode.DoubleRow,
                        )
                        k += 1
                return ps

            o2cur = [None] * B_PER_CORE

            def emit_drain(b, t, ps_tiles):
                nonlocal ndrain, nstore
                if t % STG == 0:
                    o2cur[b] = o2p.tile(
                        [128, STG, F], BF16, tag="o2", name=f"o2_{b}_{t}"
                    )
                o2t = o2cur[b]
                ps = ps_tiles[t // 2]
                bc = betaCol[:, ds(b * N_TILES + t, 1)]
                if ndrain % 2 == 0:
                    nc.scalar.mul(o2t[:, t % STG, :], ps[:, t % 2, :], bc)
                else:
                    nc.vector.tensor_scalar_mul(o2t[:, t % STG, :], ps[:, t % 2, :], bc)
                ndrain += 1
                if t % STG == STG - 1 or t == N_TILES - 1:
                    n = t % STG + 1
                    g0 = t - n + 1
                    store_q[nstore % 2].dma_start(
                        o_d[ds((b * N_TILES + g0) * 128, n * 128), :].rearrange(
                            "(t p) f -> p t f", t=n
                        ),
                        o2t[:, :n, :],
                    )
                    nstore += 1

            # ---------- interleaved two-image pipeline ----------
            ps_tiles = [[] for _ in range(B_PER_CORE)]  # per-image psum tiles
            bq = [0] * B_PER_CORE       # next beta seg
            betac = [0] * B_PER_CORE    # tiles covered by betaCol
            mm_p = [0] * B_PER_CORE     # next px-tile pair to emit
            dr_t = [0] * B_PER_CORE     # next px-tile to drain
            scols_v = [0] * B_PER_CORE
            pcov_v = [0] * B_PER_CORE
            for c in range(7):
                for b in range(B_PER_CORE):
                    t0, nt = XPM_CH[c]
                    emit_xpm_chunk(b, t0, nt)
                    scols_v[b] = S_SEG if c == 6 else 14 * (c + 1)
                    c0, npix = XT_CH[c]
                    emit_xt_chunk(b, c0, npix)
                    pcov_v[b] = c0 + npix
                progressed = True
                while progressed:
                    progressed = False
                    for b in range(B_PER_CORE):
                        while (
                            bq[b] < len(BSEGS)
                            and BSEGS[bq[b]][0] + BSEGS[bq[b]][1] + 2 <= scols_v[b]
                        ):
                            t0, cn = BSEGS[bq[b]]
                            emit_beta_seg(b, t0, cn)
                            betac[b] = t0 + cn
                            bq[b] += 1
                            progressed = True
                        if mm_p[b] < N_PAIRS:
                            p = mm_p[b]
                            last_t = min(2 * p + 1, N_TILES - 1)
                            # planes reach px base+353 (max tap offset 226+127)
                            need_px = 128 * last_t + 354
                            if need_px <= pcov_v[b]:
                                ps_tiles[b].append(emit_mm_pair(b, p))
                                mm_p[b] += 1
                                progressed = True
                        while (
                            dr_t[b] < min(2 * mm_p[b], N_TILES)
                            and dr_t[b] < betac[b]
                        ):
                            emit_drain(b, dr_t[b], ps_tiles[b])
                            dr_t[b] += 1
                            progressed = True
            assert bq == [len(BSEGS)] * B_PER_CORE, bq
            assert mm_p == [N_PAIRS] * B_PER_CORE, mm_p
            assert dr_t == [N_TILES] * B_PER_CORE, dr_t

    nc.compile()
    return nc


_NC = None


def _get_nc():
    global _NC
    if _NC is None:
        _NC = build_nc()
    return _NC


def _quantize_weights(kernels, alphas):
    """Fold estimators, then per-channel-scale fp8 e4m3 quantization."""
    sgn = np.where(kernels >= 0, 1.0, -1.0).astype(np.float32)  # [E,3,3,128,256]
    W = np.einsum("ehwcf,ef->hwcf", sgn, alphas.astype(np.float32))  # [3,3,128,256]
    # scale search on the single-term taps; tap (2,2) is two-term (near exact)
    Wf = np.stack([W[kh, kw] for kh, kw in SLOT_TAPS]).reshape(8 * CIN, F)
    scales = np.geomspace(6.0, 100.0, 385).astype(np.float32)
    q = (Wf[None, :, :] * scales[:, None, None]).astype(ml_dtypes.float8_e4m3fn)
    err = ((q.astype(np.float32) / scales[:, None, None] - Wf[None]) ** 2).sum(axis=1)
    s = scales[np.argmin(err, axis=0)]  # [F]
    Wq = (W * s).astype(ml_dtypes.float8_e4m3fn)  # [3,3,128,256]
    Wres = W * s - Wq.astype(np.float32)
    Wq2 = Wres.astype(ml_dtypes.float8_e4m3fn)  # second term for tap (2,2)
    # slot pairs (2i, 2i+1) follow PAIR_OFF via SLOT_TAPS; tap (2,2) two-term
    w8 = np.zeros((CIN, 2 * NPAIR, F), ml_dtypes.float8_e4m3fn)
    for j, (kh, kw) in enumerate(SLOT_TAPS):
        w8[:, j, :] = Wq[kh, kw]
    w8[:, 8, :] = Wq[2, 2]
    w8[:, 9, :] = Wq2[2, 2]
    sinv = (1.0 / s).astype(np.float32)  # [F]; applied on host after gather
    return np.ascontiguousarray(w8.reshape(CIN, 2 * NPAIR * F)), sinv


def _in_maps(x, kernels, alphas):
    x = np.asarray(x, np.float32)
    kernels = np.asarray(kernels, np.float32)
    alphas = np.asarray(alphas, np.float32)
    w8, sinv = _quantize_weights(kernels, alphas)
    boxm = _box_matrices()

    xb = x.astype(ml_dtypes.bfloat16)  # sign-exact; |x| sums lose <0.1%
    xs = xb.reshape(N_CORES, B_PER_CORE, IMG_PIX, CIN)
    maps = []
    for c in range(N_CORES):
        xT = np.zeros((CIN, B_PER_CORE * IMG_PAD), ml_dtypes.bfloat16)
        for b in range(B_PER_CORE):
            xT[:, b * IMG_PAD : b * IMG_PAD + IMG_PIX] = xs[c, b].T
        xpm = np.ascontiguousarray(
            xs[c].reshape(B_PER_CORE, S_TILES, 128, CIN)
            .transpose(2, 0, 1, 3)
            .reshape(128, B_PER_CORE * S_TILES, CIN)
        )
        maps.append(
            {
                "xT": np.ascontiguousarray(xT),
                "xpm": xpm,
                "w8": w8,
                "boxm": boxm,
            }
        )
    return maps, sinv


def _gather(results, sinv):
    outs = []
    sv = sinv.reshape(1, 1, 1, F)
    for c in range(N_CORES):
        o = np.asarray(results[c]["out"]).astype(np.float32)  # [24832, 256]
        o = o.reshape(B_PER_CORE, STAGE_PIX, F)[:, :OUT_PIX]
        o = o.reshape(B_PER_CORE, OUT_ROWS, W_IMG, F)[:, :, :OUT_ROWS] * sv
        outs.append(o)
    return np.ascontiguousarray(np.concatenate(outs, axis=0))


def kernel(x, kernels, alphas):
    nc = _get_nc()
    maps, sinv = _in_maps(x, kernels, alphas)
    res = run_bass_kernel_spmd(nc, maps, core_ids=list(range(N_CORES)))
    return _gather(res.results, sinv)


def _install_profile_hook():
    """The agent image's antenv lacks axon_hooks; recreate it so
    run_bass_kernel_spmd(trace=True) can NTFF-profile via libaxon_pjrt.so."""
    import types

    import antenv

    if "antenv.axon_hooks" in sys.modules:
        return
    mod = types.ModuleType("antenv.axon_hooks")
    holder = {}
    mod.set_axon_ntff_profile_hook = lambda h: holder.__setitem__("h", h)
    mod.get_axon_ntff_profile_hook = lambda: holder.get("h")
    sys.modules["antenv.axon_hooks"] = mod
    antenv.axon_hooks = mod

    from trn_agent_boot.trn_boot import _ntff_profile_via_ctypes

    hook = _ntff_profile_via_ctypes("/opt/axon/libaxon_pjrt.so")
    mod.set_axon_ntff_profile_hook(hook)

    # upload_artifacts wants a cloud bucket; keep everything local instead.
    import concourse.bass_utils as bu

    bu.upload_artifacts = lambda tmpdir: tmpdir


def run_profiled(x, kernels, alphas, tmpdir=None):
    """Returns (output, exec_time_ns, profile_json_path)."""
    _install_profile_hook()
    nc = _get_nc()
    maps, sinv = _in_maps(x, kernels, alphas)
    res = run_bass_kernel_spmd(
        nc,
        maps,
        core_ids=list(range(N_CORES)),
        trace=True,
        tmpdir=tmpdir,
    )
    return _gather(res.results, sinv), res.exec_time_ns, res.profile_json
